# revision 1
# baseline (speedup 1.0000x reference)
"""Trainium2 kernel for nn_Non_LinearGNN: 8-core SPMD Bass kernel.

Device (8 NeuronCores, edge-sharded): the FLOP-dominant per-edge MLPs
(Xi: 16 fused 64-dim layers -> A;  Rou trunk: 6 fused 32-dim layers -> f2),
bf16 matmuls with fp32 PSUM accumulation, residuals folded into PE via
identity-matmul accumulation, bias+PReLU fused into one ScalarE activation
per layer, even/odd chunk pairs packed into PE array quadrants.

Host: index gathers (featT[X_Node], featT[X_Neis]), the tiny 8-dim S-layers,
message-passing segment-sum (bincount), readout MLP + BatchNorm, output
assembly.
"""

import os
import sys

import numpy as np

sys.path.insert(0, "/opt/trn_rl_repo")

import ml_dtypes

import concourse.bacc as bacc
import concourse.mybir as mybir
import concourse.tile as tile
from concourse import bass_utils

BF16 = ml_dtypes.bfloat16

V = 50000
E = 400000
LN = 32
S = 8
ITER = 2
DEG = 8.0
MU = 0.8
D = LN + S
EPS = 1e-5
NCORES = 8
EC = E // NCORES          # real edges per core
EPAD = 53248              # padded edges per core (52 chunk-pairs of 1024)
COLS2 = EPAD // 2         # 26624 columns in 2-packed layout
COLS4 = EPAD // 4         # 13312 columns in 4-packed layout
ABLK = 2048               # ACT block columns (4 PSUM banks)

_CACHE = {}
LAST_RESULT = {}


def _build_nc():
    nc = bacc.Bacc("TRN2", target_bir_lowering=False, debug=False)
    dt = mybir.dt

    xp_d = nc.declare_dram_parameter("xp", [128, COLS2], dt.bfloat16, isOutput=False)
    xr_d = nc.declare_dram_parameter("xr", [128, COLS4], dt.bfloat16, isOutput=False)
    wx_d = nc.declare_dram_parameter("wx", [128, 64 * 5], dt.bfloat16, isOutput=False)
    wr_d = nc.declare_dram_parameter("wr", [128, 128 + 32], dt.bfloat16, isOutput=False)
    bx_d = nc.declare_dram_parameter("bx", [128, 6], dt.float32, isOutput=False)
    at_d = nc.declare_dram_parameter("at", [128, COLS2], dt.bfloat16, isOutput=True)
    f2_d = nc.declare_dram_parameter("f2r", [128, COLS4], dt.bfloat16, isOutput=True)

    AF = mybir.ActivationFunctionType
    with tile.TileContext(nc) as tc:
        with (
            tc.tile_pool(name="big", bufs=1) as big,
            tc.tile_pool(name="wpool", bufs=1) as wp,
            tc.tile_pool(name="ps", bufs=2, space="PSUM") as pp,
        ):
            xp = big.tile([128, COLS2], dt.bfloat16, tag="xp")
            f2t = big.tile([128, COLS2], dt.bfloat16, tag="f2")
            ht = big.tile([128, COLS2], dt.bfloat16, tag="h")
            wx = wp.tile([128, 64 * 5], dt.bfloat16, tag="wx")
            wr = wp.tile([128, 128 + 32], dt.bfloat16, tag="wr")
            bx = wp.tile([128, 6], dt.float32, tag="bx")
            nc.sync.dma_start(wx[:], wx_d[:, :])
            nc.sync.dma_start(wr[:], wr_d[:, :])
            nc.sync.dma_start(bx[:], bx_d[:, :])
            nc.sync.dma_start(xp[:], xp_d[:, :])

            def wslot(i):
                return wx[:, 64 * i:64 * (i + 1)]

            # Xi layer schedule: (wslot, bias_col, residual_tile or None, out)
            # slots: 0=xi1 (W top / I64 bottom), 1=xi2, 2=xi3, 3=xi3a
            sched = []
            for _ in range(5):
                sched.append((0, 0, "xp", "h"))
            sched.append((1, 1, None, "f2"))
            for i in range(5):
                sched.append((2, 2, None, "h"))
                sched.append((3, 3, "f2", "at" if i == 4 else "h"))

            h = xp  # layer-0 input is X itself
            nblk = COLS2 // ABLK
            for (slot, bcol, res, outk) in sched:
                w = wslot(slot)
                bias = bx[:, bcol:bcol + 1]
                hn = f2t if outk == "f2" else ht
                rest = xp if res == "xp" else (f2t if res == "f2" else None)
                for b in range(nblk):
                    ps = pp.tile([128, ABLK], dt.float32, tag="psA")
                    for q in range(ABLK // 512):
                        c0 = b * ABLK + q * 512
                        cs = slice(c0, c0 + 512)
                        qs = slice(q * 512, (q + 1) * 512)
                        nc.tensor.matmul(ps[0:64, qs], w[0:64, :], h[0:64, cs],
                                         start=True, stop=(rest is None),
                                         tile_position=(0, 0))
                        if rest is not None:
                            nc.tensor.matmul(ps[0:64, qs], wx[0:64, 256:320],
                                             rest[0:64, cs], start=False,
                                             stop=True, tile_position=(0, 0))
                        nc.tensor.matmul(ps[64:128, qs], w[64:128, :], h[64:128, cs],
                                         start=True, stop=(rest is None),
                                         tile_position=(64, 64))
                        if rest is not None:
                            nc.tensor.matmul(ps[64:128, qs], wx[64:128, 256:320],
                                             rest[64:128, cs], start=False,
                                             stop=True, tile_position=(64, 64))
                    bs = slice(b * ABLK, (b + 1) * ABLK)
                    nc.scalar.activation(hn[:, bs], ps[:], AF.Prelu,
                                         bias=bias, scale=1.0, alpha=0.25)
                h = hn
                if outk == "at":
                    nc.sync.dma_start(at_d[:, :], hn[:])

            # ---- Rou trunk: 5 residual 32-dim layers + r2 (32->8), 4-packed
            # xr reuses the xp slot (xp is dead after Xi layer 5; Tile's WAR
            # tracking orders the DMA after the last xp read)
            xr = big.tile([128, COLS4], dt.bfloat16, tag="xp")
            nc.sync.dma_start(xr[:], xr_d[:, :])
            rht = big.tile([128, COLS4], dt.bfloat16, tag="rh")
            rh = xr
            RBLK = 512
            rnb = COLS4 // RBLK
            for lay in range(6):
                last = lay == 5
                rn = rht
                for b in range(rnb):
                    cs = slice(b * RBLK, (b + 1) * RBLK)
                    if last:
                        ps = pp.tile([128, RBLK], dt.float32, tag="psA")
                        nc.tensor.matmul(ps[0:32, :], wr[:, 128:160], rh[:, cs],
                                         start=True, stop=True)
                        nc.scalar.activation(rn[0:32, cs], ps[0:32, :], AF.Prelu,
                                             bias=bx[0:32, 5:6], scale=1.0,
                                             alpha=0.25)
                    else:
                        ps = pp.tile([128, RBLK], dt.float32, tag="psA")
                        nc.tensor.matmul(ps[:], wr[:, 0:128], rh[:, cs],
                                         start=True, stop=False)
                        # residual += xr: two K=64 identity matmuls
                        nc.tensor.matmul(ps[0:64, :], wx[0:64, 256:320],
                                         xr[0:64, cs], start=False, stop=True,
                                         tile_position=(0, 0))
                        nc.tensor.matmul(ps[64:128, :], wx[64:128, 256:320],
                                         xr[64:128, cs], start=False, stop=True,
                                         tile_position=(64, 64))
                        nc.scalar.activation(rn[:, cs], ps[:], AF.Prelu,
                                             bias=bx[:, 4:5], scale=1.0,
                                             alpha=0.25)
                rh = rn
            nc.sync.dma_start(f2_d[:, :], rh[:, :])

    nc.compile()
    return nc


def _pack2(xt):
    """[EPAD, 64] -> [128, COLS2] bf16: even chunks lanes 0-63, odd 64-127."""
    c = xt.reshape(EPAD // 512, 512, 64)           # chunk, edge, feat
    ev = c[0::2].transpose(0, 2, 1).reshape(-1, 64, 512)
    od = c[1::2].transpose(0, 2, 1).reshape(-1, 64, 512)
    out = np.empty((128, COLS2), dtype=BF16)
    out[0:64] = np.concatenate(list(ev), axis=1)
    out[64:128] = np.concatenate(list(od), axis=1)
    return out


def _unpack2(at):
    """[128, COLS2] -> [EPAD, 64] fp32."""
    npair = EPAD // 1024
    ev = at[0:64].reshape(64, npair, 512).transpose(1, 2, 0)
    od = at[64:128].reshape(64, npair, 512).transpose(1, 2, 0)
    out = np.empty((EPAD // 512, 512, 64), dtype=np.float32)
    out[0::2] = ev
    out[1::2] = od
    return out.reshape(EPAD, 64)


def _pack4(xh):
    """[EPAD, 32] -> [128, COLS4] bf16: chunk 4g+i at lanes 32i..32i+32."""
    c = xh.reshape(EPAD // 512, 512, 32)
    out = np.empty((128, COLS4), dtype=BF16)
    for i in range(4):
        gi = c[i::4].transpose(0, 2, 1)            # [g, 32, 512]
        out[32 * i:32 * (i + 1)] = np.concatenate(list(gi), axis=1)
    return out


def _unpack4_8(f2r):
    """[128, COLS4] -> [EPAD, 8] fp32; r2 blockdiag puts group i at lanes
    8i..8i+8 (bd4(r2w.T) block i occupies output columns 8i..8i+8)."""
    ng = EPAD // 2048
    out = np.empty((EPAD // 512, 512, 8), dtype=np.float32)
    for i in range(4):
        gi = f2r[8 * i:8 * i + 8].reshape(8, ng, 512).transpose(1, 2, 0)
        out[i::4] = gi
    return out.reshape(EPAD, 8)


def _prelu(x, a):
    return np.where(x >= 0, x, a * x)


def kernel(**inputs):
    X_Node = np.asarray(inputs["X_Node"]).astype(np.int64)
    X_Neis = np.asarray(inputs["X_Neis"]).astype(np.int64)
    fM = np.asarray(inputs["feature_Matrix"], dtype=np.float32)
    H0 = np.asarray(inputs["node_states"], dtype=np.float32)
    g = {k: np.asarray(v, dtype=np.float32) for k, v in inputs.items()
         if k not in ("X_Node", "X_Neis")}

    featT = fM.T                                        # [V, LN]
    Xn = featT[X_Node]                                  # [E, 32]
    Xs = featT[X_Neis]
    X = np.concatenate([Xn, Xs], axis=1).astype(BF16)   # [E, 64] bf16

    # ---- device inputs per core
    I64 = np.eye(64, dtype=BF16)

    def bd4(w):
        out = np.zeros((4 * w.shape[0], 4 * w.shape[1]), dtype=BF16)
        for i in range(4):
            out[i * w.shape[0]:(i + 1) * w.shape[0],
                i * w.shape[1]:(i + 1) * w.shape[1]] = w.astype(BF16)
        return out

    wx = np.zeros((128, 64 * 5), dtype=BF16)
    for i, wname in enumerate(["xi1w", "xi2w", "xi3w", "xi3aw"]):
        wt = g[wname].T.astype(BF16)                    # lhsT = W.T
        wx[0:64, 64 * i:64 * i + 64] = wt
        wx[64:128, 64 * i:64 * i + 64] = wt
    wx[0:64, 256:320] = I64                             # identity for residuals
    wx[64:128, 256:320] = I64

    wr = np.zeros((128, 160), dtype=BF16)
    wr[:, 0:128] = bd4(g["r1w"].T)                      # [32,32].T blockdiag
    wr[:, 128:160] = bd4(g["r2w"].T)                    # [128, 32]

    bxv = np.zeros((128, 6), dtype=np.float32)
    for i, bn in enumerate(["xi1b", "xi2b", "xi3b", "xi3ab"]):
        bxv[0:64, i] = g[bn]
        bxv[64:128, i] = g[bn]
    bxv[:, 4] = np.tile(g["r1b"], 4)
    bxv[0:32, 5] = np.tile(g["r2b"], 4)

    in_maps = []
    for c in range(NCORES):
        sl = slice(c * EC, (c + 1) * EC)
        Xc = np.zeros((EPAD, 64), dtype=BF16)
        Xc[:EC] = X[sl]
        in_maps.append({
            "xp": _pack2(Xc),
            "xr": _pack4(Xc[:, 0:32]),
            "wx": wx, "wr": wr, "bx": bxv,
        })

    if "nc" not in _CACHE:
        _CACHE["nc"] = _build_nc()
    nc = _CACHE["nc"]

    trace = bool(int(os.environ.get("KERNEL_TRACE", "0")))
    import time as _time
    try:
        t0 = _time.time()
        res = bass_utils.run_bass_kernel_spmd(
            nc, in_maps, core_ids=list(range(NCORES)), trace=trace)
    except ModuleNotFoundError:
        t0 = _time.time()
        res = bass_utils.run_bass_kernel_spmd(
            nc, in_maps, core_ids=list(range(NCORES)), trace=False)
    LAST_RESULT["run_wall_s"] = _time.time() - t0
    LAST_RESULT["exec_time_ns"] = res.exec_time_ns

    # ---- host: unpack A, f2; S-layers; message passing; readout
    A = np.empty((E, S, S), dtype=np.float32)
    b = np.empty((E, S), dtype=np.float32)
    for c in range(NCORES):
        sl = slice(c * EC, (c + 1) * EC)
        h16 = _unpack2(res.results[c]["at"])[:EC]
        A[sl] = (MU / S / DEG) * h16.reshape(EC, S, S)
        f2 = _unpack4_8(res.results[c]["f2r"])[:EC]
        h = f2.copy()
        for _ in range(5):
            h = _prelu(h @ g["r3aw"].T + g["r3ab"] + f2, g["ra"])
        b[sl] = h

    H = H0
    for _ in range(ITER):
        Hb = H.astype(BF16).astype(np.float32)
        msg = np.einsum("eij,ej->ei", A.astype(BF16).astype(np.float32),
                        Hb[X_Neis]) + b
        H = np.stack([np.bincount(X_Node, weights=msg[:, i], minlength=V)
                      for i in range(S)], axis=1).astype(np.float32)

    out = np.concatenate([featT, H], axis=1)
    o = out
    for _ in range(10):
        o = _prelu(o @ g["l1w"].T + g["l1b"], g["ga"])
        o = o @ g["l1aw"].T + g["l1ab"]
        o = _prelu(o + out, g["ga"])
        mean = o.mean(0)
        var = ((o - mean) ** 2).mean(0)
        o = (o - mean) / np.sqrt(var + EPS) * g["bn_g"] + g["bn_b"]
    o = o @ g["l2w"].T + g["l2b"]
    o2 = _prelu(o, g["ga"])
    o3 = o2
    for _ in range(10):
        o3 = _prelu(o3 @ g["l3w"].T + g["l3b"], g["ga"])
        o3 = o3 @ g["l3aw"].T + g["l3ab"]
        o3 = _prelu(o3 + o2, g["ga"])
    return np.concatenate([o3[:, 0], o3[:, 1]], axis=0).astype(np.float32)

